# Initial kernel scaffold
#
"""Per-row asymmetric int4 quantization (QuaRot asym_quant) on 8 TRN2 cores.

Full input x: [16384, 4096] f32. Outputs: q [16384,4096] f32,
scale [16384,1] f32, zeros [16384,1] f16.

Sharding: trivially data-parallel along rows — 2048 rows per core, no
communication. Per core: 16 tiles of [128 partitions, 4096 free].

Per-tile math (row r on partition p):
  mx = max(x_r); nm = -min(x_r)            (DVE tensor_tensor_reduce over
                                            the two row halves: elementwise
                                            max/min of halves fused with a
                                            free-axis reduce -> half cycles)
  d = mx + nm; scale = d * (1/15); a = 1/scale   (DVE reciprocal = HW divide)
  zb = RN(nm*a + MAGIC) = MAGIC + round(-min/scale) = MAGIC + zeros
       (ACT fma; MAGIC = 1.5*2^23 so RN lands on the integer grid, half-even
        like jnp.round)
  v  = RN(x*a + zb) = MAGIC + round(x/scale) + zeros   (ACT big pass)
  v  = min(max(v, MAGIC), MAGIC+15)        (DVE tensor_scalar, both clamps)
  q  = v - MAGIC                           (ACT, Sterbenz-exact)
"""

import numpy as np

import concourse.bass as bass
import concourse.tile as tile
from concourse import mybir
from concourse.bass_utils import run_bass_kernel_spmd

N_CORES = 8
R_FULL, C = 16384, 4096
R = R_FULL // N_CORES  # rows per core
P = 128                # partitions per tile
NT = R // P            # tiles per core
HALF = C // 2
MAXQ = 15.0
MAGIC = 12582912.0     # 1.5 * 2**23: RN(t + MAGIC) == MAGIC + round_half_even(t)
F32_LOWEST = -3.4028234663852886e38

_cached = {}


def build_nc() -> bass.Bass:
    nc = bass.Bass("TRN2", target_bir_lowering=False)
    x = nc.dram_tensor("x", [R, C], mybir.dt.float32, kind="ExternalInput").ap()
    q = nc.dram_tensor("q", [R, C], mybir.dt.float32, kind="ExternalOutput").ap()
    s = nc.dram_tensor("scale", [R, 1], mybir.dt.float32, kind="ExternalOutput").ap()
    z = nc.dram_tensor("zeros", [R, 1], mybir.dt.float16, kind="ExternalOutput").ap()

    xt = x.rearrange("(n p) c -> n p c", p=P)
    qt = q.rearrange("(n p) c -> n p c", p=P)
    st = s.rearrange("(n p) o -> n p o", p=P)
    zt = z.rearrange("(n p) o -> n p o", p=P)

    with tile.TileContext(nc) as tc:
        with (
            tc.tile_pool(name="xp", bufs=3) as xp,
            tc.tile_pool(name="vp", bufs=3) as vp,
            tc.tile_pool(name="sm", bufs=4) as sm,
        ):
            for i in range(NT):
                xtile = xp.tile([P, C], mybir.dt.float32)
                nc.sync.dma_start(out=xtile, in_=xt[i])

                mx = sm.tile([P, 1], mybir.dt.float32, tag="mx")
                nm = sm.tile([P, 1], mybir.dt.float32, tag="nm")
                dummy = sm.tile([P, 1], mybir.dt.float32, tag="dummy")
                dummy2 = sm.tile([P, 1], mybir.dt.float32, tag="dummy2")
                # row max: elementwise max of the two halves, reduced with max
                nc.vector.tensor_tensor_reduce(
                    out=dummy.broadcast_to([P, HALF]),
                    in0=xtile[:, :HALF],
                    in1=xtile[:, HALF:],
                    scale=1.0,
                    scalar=F32_LOWEST,
                    op0=mybir.AluOpType.max,
                    op1=mybir.AluOpType.max,
                    accum_out=mx,
                )
                # negated row min: (min of halves) * -1, reduced with max
                nc.vector.tensor_tensor_reduce(
                    out=dummy2.broadcast_to([P, HALF]),
                    in0=xtile[:, :HALF],
                    in1=xtile[:, HALF:],
                    scale=-1.0,
                    scalar=F32_LOWEST,
                    op0=mybir.AluOpType.min,
                    op1=mybir.AluOpType.max,
                    accum_out=nm,
                )
                d = sm.tile([P, 1], mybir.dt.float32, tag="d")
                nc.vector.tensor_tensor(out=d, in0=mx, in1=nm, op=mybir.AluOpType.add)
                s_t = sm.tile([P, 1], mybir.dt.float32, tag="s_t")
                nc.vector.tensor_scalar_mul(s_t, d, 1.0 / MAXQ)
                a = sm.tile([P, 1], mybir.dt.float32, tag="a")
                nc.vector.reciprocal(out=a, in_=s_t)
                # zb = MAGIC + round(nm * a) = MAGIC + zeros
                zb = sm.tile([P, 1], mybir.dt.float32, tag="zb")
                nc.scalar.activation(
                    out=zb,
                    in_=nm,
                    func=mybir.ActivationFunctionType.Identity,
                    bias=MAGIC,
                    scale=a,
                )
                zf = sm.tile([P, 1], mybir.dt.float16, tag="zf")
                nc.vector.tensor_scalar_sub(zf, zb, MAGIC)

                # big pass: v = RN(x*a + zb) -> MAGIC + round(x/scale) + zeros
                v = vp.tile([P, C], mybir.dt.float32)
                nc.scalar.activation(
                    out=v,
                    in_=xtile,
                    func=mybir.ActivationFunctionType.Identity,
                    bias=zb,
                    scale=a,
                )
                # both clamps in one DVE pass (2x mode: single-src f32 SBUF)
                nc.vector.tensor_scalar(
                    out=v,
                    in0=v,
                    scalar1=MAGIC,
                    scalar2=MAGIC + MAXQ,
                    op0=mybir.AluOpType.max,
                    op1=mybir.AluOpType.min,
                )
                # subtract MAGIC (exact) on ACT
                nc.scalar.activation(
                    out=v,
                    in_=v,
                    func=mybir.ActivationFunctionType.Identity,
                    bias=-MAGIC,
                    scale=1.0,
                )

                nc.sync.dma_start(out=qt[i], in_=v)
                nc.sync.dma_start(out=st[i], in_=s_t)
                nc.sync.dma_start(out=zt[i], in_=zf)
    return nc


def _get_nc() -> bass.Bass:
    if "nc" not in _cached:
        _cached["nc"] = build_nc()
    return _cached["nc"]


def kernel(x: np.ndarray, **_unused) -> tuple[np.ndarray, np.ndarray, np.ndarray]:
    x = np.ascontiguousarray(np.asarray(x, dtype=np.float32))
    assert x.shape == (R_FULL, C), x.shape
    nc = _get_nc()
    in_maps = [
        {"x": np.ascontiguousarray(x[i * R : (i + 1) * R])} for i in range(N_CORES)
    ]
    res = run_bass_kernel_spmd(nc, in_maps, core_ids=list(range(N_CORES)))
    q = np.concatenate([res.results[i]["q"] for i in range(N_CORES)], axis=0)
    scale = np.concatenate([res.results[i]["scale"] for i in range(N_CORES)], axis=0)
    zeros = np.concatenate([res.results[i]["zeros"] for i in range(N_CORES)], axis=0)
    return q, scale.astype(np.float32), zeros.astype(np.float16)


# revision 6
# speedup vs baseline: 4.1957x; 4.1957x over previous
"""Per-row asymmetric int4 quantization (QuaRot asym_quant) on 8 TRN2 cores.

Full input x: [16384, 4096] f32. Outputs: q [16384,4096] f32,
scale [16384,1] f32, zeros [16384,1] f16.

Sharding: trivially data-parallel along rows — 2048 rows per core, no
communication. Per core: 16 tiles of [128 partitions, 4096 free].

Per-tile math (row r on partition p):
  mx = max(x_r); nm = -min(x_r)            (DVE tensor_scalar with fused
                                            max-reduce accumulator)
  d = mx + nm; scale = d * (1/15); a = 1/scale   (DVE reciprocal = HW divide)
  zb = RN(nm*a + MAGIC) = MAGIC + round(-min/scale) = MAGIC + zeros
       (ACT fma; MAGIC = 1.5*2^23 so RN lands on the integer grid, half-even
        like jnp.round)
  v  = RN(x*a + zb) = MAGIC + round(x/scale) + zeros   (ACT big pass)
  v  = min(max(v, MAGIC), MAGIC+15)        (DVE tensor_scalar, both clamps)
  q  = v - MAGIC                           (ACT, Sterbenz-exact)

DMA discipline: in-DMAs issue from the Sync sequencer, out-DMAs from the
Scalar sequencer (two separate HWDGE rings) so a blocked out-DMA never
stalls the issue of the next tile's load. scale/zeros are staged in
[128, NT] SBUF tiles and written with one DMA each at the end (DRAM
layout [P, NT], transposed to [R, 1] on the host).
"""

import numpy as np

import concourse.bacc as bacc
import concourse.bass as bass
import concourse.tile as tile
from concourse import mybir
from concourse.bass_utils import run_bass_kernel_spmd

N_CORES = 8
R_FULL, C = 16384, 4096
R = R_FULL // N_CORES  # rows per core
P = 128                # partitions per tile
NT = R // P            # tiles per core
MAXQ = 15.0
MAGIC = 12582912.0     # 1.5 * 2**23: RN(t + MAGIC) == MAGIC + round_half_even(t)

_cached = {}


def build_nc() -> bass.Bass:
    nc = bacc.Bacc("TRN2", target_bir_lowering=False)
    x = nc.dram_tensor("x", [R, C], mybir.dt.float32, kind="ExternalInput").ap()
    q = nc.dram_tensor("q", [R, C], mybir.dt.float32, kind="ExternalOutput").ap()
    # staged [P, NT]: column j = tile j's per-partition value; host transposes
    s = nc.dram_tensor("scale", [P, NT], mybir.dt.float32, kind="ExternalOutput").ap()
    z = nc.dram_tensor("zeros", [P, NT], mybir.dt.float16, kind="ExternalOutput").ap()

    xt = x.rearrange("(n p) c -> n p c", p=P)
    qt = q.rearrange("(n p) c -> n p c", p=P)

    with tile.TileContext(nc) as tc:
        with (
            tc.tile_pool(name="xp", bufs=4) as xp,
            tc.tile_pool(name="vp", bufs=3) as vp,
            tc.tile_pool(name="sm", bufs=4) as sm,
            tc.tile_pool(name="singles", bufs=1) as singles,
        ):
            magic_b = singles.tile([P, 1], mybir.dt.float32, tag="magic_b")
            nc.vector.memset(magic_b, MAGIC)
            neg_magic_b = singles.tile([P, 1], mybir.dt.float32, tag="neg_magic_b")
            nc.vector.memset(neg_magic_b, -MAGIC)
            s_acc = singles.tile([P, NT], mybir.dt.float32, tag="s_acc")
            z_acc = singles.tile([P, NT], mybir.dt.float16, tag="z_acc")
            for i in range(NT):
                xtile = xp.tile([P, C], mybir.dt.float32)
                nc.sync.dma_start(out=xtile, in_=xt[i])

                mx = sm.tile([P, 1], mybir.dt.float32, tag="mx")
                nm = sm.tile([P, 1], mybir.dt.float32, tag="nm")
                dummy = sm.tile([P, 1], mybir.dt.float32, tag="dummy")
                dummy2 = sm.tile([P, 1], mybir.dt.float32, tag="dummy2")
                # row max: tensor_scalar bypass with fused max-reduce
                nc.vector.tensor_scalar(
                    out=dummy.broadcast_to([P, C]),
                    in0=xtile,
                    scalar1=0.0,
                    scalar2=None,
                    op0=mybir.AluOpType.bypass,
                    op1=mybir.AluOpType.max,
                    accum_out=mx,
                )
                # negated row min: (x * -1) max-reduced
                nc.vector.tensor_scalar(
                    out=dummy2.broadcast_to([P, C]),
                    in0=xtile,
                    scalar1=-1.0,
                    scalar2=None,
                    op0=mybir.AluOpType.mult,
                    op1=mybir.AluOpType.max,
                    accum_out=nm,
                )
                d = sm.tile([P, 1], mybir.dt.float32, tag="d")
                nc.vector.tensor_tensor(out=d, in0=mx, in1=nm, op=mybir.AluOpType.add)
                # scale for this tile -> staging column i
                nc.vector.tensor_scalar_mul(s_acc[:, i : i + 1], d, 1.0 / MAXQ)
                a = sm.tile([P, 1], mybir.dt.float32, tag="a")
                nc.vector.reciprocal(out=a, in_=s_acc[:, i : i + 1])
                # zb = MAGIC + round(nm * a) = MAGIC + zeros
                zb = sm.tile([P, 1], mybir.dt.float32, tag="zb")
                nc.scalar.activation(
                    out=zb,
                    in_=nm,
                    func=mybir.ActivationFunctionType.Identity,
                    bias=magic_b,
                    scale=a,
                )
                # zeros f16 -> staging column i
                nc.vector.tensor_scalar_sub(z_acc[:, i : i + 1], zb, MAGIC)

                # big pass: v = RN(x*a + zb) -> MAGIC + round(x/scale) + zeros
                v = vp.tile([P, C], mybir.dt.float32)
                nc.scalar.activation(
                    out=v,
                    in_=xtile,
                    func=mybir.ActivationFunctionType.Identity,
                    bias=zb,
                    scale=a,
                )
                # both clamps in one DVE pass (2x mode: single-src f32 SBUF)
                nc.vector.tensor_scalar(
                    out=v,
                    in0=v,
                    scalar1=MAGIC,
                    scalar2=MAGIC + MAXQ,
                    op0=mybir.AluOpType.max,
                    op1=mybir.AluOpType.min,
                )
                # subtract MAGIC (exact) on ACT
                nc.scalar.activation(
                    out=v,
                    in_=v,
                    func=mybir.ActivationFunctionType.Identity,
                    bias=neg_magic_b,
                    scale=1.0,
                )

                # out-DMA on the Scalar HWDGE ring (keeps Sync ring free for loads)
                nc.scalar.dma_start(out=qt[i], in_=v)
            nc.scalar.dma_start(out=s, in_=s_acc)
            nc.scalar.dma_start(out=z, in_=z_acc)
    nc.finalize()
    return nc


def _get_nc() -> bass.Bass:
    if "nc" not in _cached:
        _cached["nc"] = build_nc()
    return _cached["nc"]


def kernel(x: np.ndarray, **_unused) -> tuple[np.ndarray, np.ndarray, np.ndarray]:
    x = np.ascontiguousarray(np.asarray(x, dtype=np.float32))
    assert x.shape == (R_FULL, C), x.shape
    nc = _get_nc()
    in_maps = [
        {"x": np.ascontiguousarray(x[i * R : (i + 1) * R])} for i in range(N_CORES)
    ]
    res = run_bass_kernel_spmd(nc, in_maps, core_ids=list(range(N_CORES)))
    q = np.concatenate([res.results[i]["q"] for i in range(N_CORES)], axis=0)
    # staged [P, NT] -> [R, 1]: row j*P + p = staged[p, j]
    scale = np.concatenate(
        [res.results[i]["scale"].T.reshape(R, 1) for i in range(N_CORES)], axis=0
    )
    zeros = np.concatenate(
        [res.results[i]["zeros"].T.reshape(R, 1) for i in range(N_CORES)], axis=0
    )
    return q, scale.astype(np.float32), zeros.astype(np.float16)
